# revision 23
# baseline (speedup 1.0000x reference)
"""Trainium2 Bass kernel for nn_MultiHeadAttention_63591285785308 (v2).

Reference semantics (faithful "reshape, no transpose" head split):
  Q = query @ Wq.T + bq            [B, S, D]
  Qh = Q.reshape(B, H, S, dk)       # head h <-> rows h*128:(h+1)*128 of Q[b]
  scores = Qh @ Kh^T / sqrt(dk); P = softmax(scores); ctx = P @ Vh
  out = ctx.reshape(B, S, D) @ Wo.T + bo

32 (b, h) units sharded 4-per-core across 8 cores.  Head-seq positions use
the j-major permutation p = j*128 + rr (true s' = 16*rr + j); attention is
permutation-invariant along s, and the out gather un-permutes for free.

v2 layout decisions (cost-model-driven):
  - QK^T in [t-part, q-free] orientation (lhsT=khT chunk, rhs=qhT).
  - PV in [q-part, dk-free] orientation: ctx[q,d] accumulates over 16
    t-chunks with lhsT = P^T chunk.  Output free dim is 65 (dk + a ones
    column of V giving the softmax denominator for free), half the cost of
    the [dk-part, q-free] orientation under the free-dim cost model.
  - Softmax denominators divided out on DVE with a free-dim broadcast.
  - All transposes (V chunks to [t-part, d-free]; ctx chunks to the
    output-projection lhsT layout) done by DMA XBAR transpose, SBUF->SBUF.
  - Q/K/V biases folded into the DVE PSUM->SBUF reshape copies.
  - Instruction emission is pace-driven: QK tiles are interleaved with
    projection / PV / output-projection work so the scalar engine's exp
    stream (the second-tightest resource) never starves.
"""

import json
from collections import deque

import numpy as np
import ml_dtypes

B, S, D, H, DK, P = 2, 2048, 1024, 16, 64, 128
NCORES = 8
UPC = 4  # units per core

# ---- packed input blob layout (bf16 columns of a [P, BLOB_COLS] tensor) ----
# weights wq/wk/wv: 4 quarters x [P, 8, 256] = 2048 cols each
_W_OFF = {"q": 0, "k": 8192, "v": 16384}
_WO_OFF = 24576          # wo: 2 halves x [P, 8, 512] = 4096 cols each
_X_OFF = {"q": 32768, "k": 36864, "v": 40960}  # 2 pairs x [P, 8, 256] each
_B_OFF = {"q": 45056, "k": 45072, "v": 45088}  # f32 [P, 8] bit-packed as 16 cols
_BO_OFF = 45104          # bo: [1, 1024] in row 0
BLOB_COLS = 46128

_BF16 = ml_dtypes.bfloat16
_prog_cache = {}

_MAX_SYNC = 2  # this walrus build allows at most 2 sync commands per instruction

# pace-driver tunables (env-overridable for sweeps)
import os as _os
PT_BUFS = int(_os.environ.get("K_PT_BUFS", "28"))
RT_MARGIN = float(_os.environ.get("K_RT_MARGIN", "1500"))
SKIP_WIN = int(_os.environ.get("K_SKIP_W", "8"))
BIAS_START = int(_os.environ.get("K_BIAS_START", "1"))
G_BUFS = int(_os.environ.get("K_G_BUFS", "12"))
QK_COST_T = float(_os.environ.get("K_QK_COST", "240"))
EXP_COST_T = float(_os.environ.get("K_EXP_COST", "1038"))


def _legalize_bir_sync(bir_bytes):
    """Split sync waits/updates exceeding the per-instruction cap onto
    adjacent same-engine NoOps (engine program order preserves semantics:
    waits move to preceding nops, update overflow to trailing nops)."""
    d = json.loads(bir_bytes)
    ctr = [0]

    def nop(engine, debug, waits, upds):
        ctr[0] += 1
        return {
            "debug": debug,
            "engine": engine,
            "ins": [],
            "name": f"I-lgl{ctr[0]}",
            "opcode": "NoOp",
            "outs": [],
            "sync_info": {"on_update": upds, "on_wait": waits},
        }

    changed = False
    for fn in d["functions"]:
        for blk in fn["blocks"]:
            new = []
            for ins in blk["instructions"]:
                si = ins.get("sync_info") or {}
                waits = list(si.get("on_wait") or [])
                upds = list(si.get("on_update") or [])
                if len(waits) + len(upds) <= _MAX_SYNC:
                    new.append(ins)
                    continue
                changed = True
                keep_u = upds[:_MAX_SYNC]
                extra_u = upds[_MAX_SYNC:]
                keep_w = waits[: max(0, _MAX_SYNC - len(keep_u))]
                extra_w = waits[len(keep_w):]
                for w in extra_w:
                    new.append(nop(ins["engine"], ins.get("debug", 0), [w], []))
                si["on_wait"] = keep_w
                si["on_update"] = keep_u
                ins["sync_info"] = si
                new.append(ins)
                for uu in extra_u:
                    new.append(nop(ins["engine"], ins.get("debug", 0), [], [uu]))
            blk["instructions"] = new
    if not changed:
        return bir_bytes
    return json.dumps(d).encode()


def _install_bir_legalizer():
    if _prog_cache.get("legalizer_installed"):
        return
    from concourse import bass2jax

    orig = bass2jax.compile_bir_kernel

    def patched(ant_bir_str, compile_dir, neff_name="file.neff", **kw):
        return orig(_legalize_bir_sync(ant_bir_str), compile_dir, neff_name=neff_name, **kw)

    bass2jax.compile_bir_kernel = patched
    _prog_cache["legalizer_installed"] = True


def _build_program(_debug_dumps=False):
    import concourse.bass as bass
    import concourse.mybir as mybir
    import concourse.tile as tile
    from concourse.vector_clock import ScopedClock, VectorClock
    from concourse.masks import make_identity

    dt = mybir.dt
    BF = dt.bfloat16
    F32 = dt.float32
    F8 = dt.float8e4
    DR = mybir.MatmulPerfMode.DoubleRow
    ADD = mybir.AluOpType.add
    MUL = mybir.AluOpType.mult
    EXP = mybir.ActivationFunctionType.Exp

    class SplitDrainTileContext(tile.TileContext):
        """This walrus build caps sem waits per instruction below what the
        stock tail drain needs; split the waits across single-wait SP nops
        (SP program order then gates the bare drain)."""

        def _drain_and_barrier(self, tick_clock, wait_clock):
            gc = tick_clock.global_clock
            for proc in range(len(gc)):
                tick = gc[proc]
                if tick <= 0:
                    continue
                vc = VectorClock()
                vc.require_at_least(proc, tick)
                nop = self.nc.sync.nop(nofuse=True)
                wait_clock.add_sem_waits(nop.ins, ScopedClock({None: vc}))
            self.nc.sync.drain()
            self.nc.all_engine_barrier()
            assert self.sems is not None
            popped = self.nc._tile_sem_poison_stack.pop()
            assert popped is self._sem_poison
            self.nc.clear_and_free_semaphores(list(self.sems.allocated().values()))
            self.nc.all_engine_barrier()

    nc = bass.Bass()

    # Single packed input blob: per-exec dispatch overhead is ~50us PER
    # INPUT BUFFER (measured), so everything ships in one tensor.  Layout
    # is pre-arranged host-side so every DMA is a clean [P, width] slice.
    blob_d = nc.declare_dram_parameter("blob", [P, BLOB_COLS], BF, isOutput=False)
    out_d = nc.declare_dram_parameter("out", [UPC, P, D], BF, isOutput=True)

    with SplitDrainTileContext(nc) as tc:
        with (
            tc.tile_pool(name="persist", bufs=1) as pp,
            tc.tile_pool(name="pt", bufs=(22 if _debug_dumps else PT_BUFS)) as ptpool,
            tc.tile_pool(name="ctxg", bufs=4) as cgpool,
            tc.tile_pool(name="g", bufs=G_BUFS) as gpool,
            tc.tile_pool(name="rec", bufs=4) as recpool,
            tc.tile_pool(name="ostage", bufs=4) as opool,
            tc.tile_pool(name="wide_ps", bufs=2, space="PSUM") as wps,
            tc.tile_pool(name="np_ps", bufs=2, space="PSUM") as nps,
            tc.tile_pool(name="ctx_ps", bufs=2, space="PSUM") as cps,
        ):
            # ---------- persistent SBUF tiles ----------
            w_sb = {}  # w_sb[nm][quarter]: [P, 8, 256] (dout quarter), wo: halves
            for nm in ("q", "k", "v"):
                w_sb[nm] = [pp.tile([P, 8, 256], BF, name=f"w_{nm}{i}", tag=f"w_{nm}{i}")
                            for i in range(4)]
            w_sb["o"] = [pp.tile([P, 8, 512], BF, name=f"w_o{i}", tag=f"w_o{i}")
                         for i in range(2)]
            x_sb = {nm: [pp.tile([P, 8, 256], BF, name=f"x_{nm}{pr}", tag=f"x_{nm}{pr}")
                         for pr in range(2)]
                    for nm in ("q", "k", "v")}
            b_sb = {}
            for nm in ("q", "k", "v"):
                b_sb[nm] = pp.tile([P, 8], F32, name=f"b_{nm}", tag=f"b_{nm}")
            bo_sb = pp.tile([1, D], BF, name="bo", tag="bo")
            ones_bf = pp.tile([1, P], BF, name="ones_bf", tag="ones_bf")

            # fp8 Q^T/K^T for DoubleRow QK, one tile per unit u = 2*pr + pu:
            # [dk 0:64 partitions, reduction-pair, head-seq].  Pair slot 0
            # holds the data, slot 1 stays zero: DR charges by output
            # columns (0.5 cyc/row), so the dead half costs nothing, and the
            # PSUM->SBUF reshape stays one [64,128] DVE op per (pu, hh).
            q8 = [pp.tile([64, 2, S], F8, name=f"q8_{u}", tag=f"q8_{u}")
                  for u in range(4)]
            k8 = [pp.tile([64, 2, S], F8, name=f"k8_{u}", tag=f"k8_{u}")
                  for u in range(4)]
            vhT = [pp.tile([P, S], BF, name=f"vhT{pr}", tag=f"vhT{pr}") for pr in range(2)]
            # vh[pr]: [t-part, tt, 2*(dk+1)]; unit pu at cols pu*65..pu*65+65,
            # with a ones column at pu*65+64 (softmax denominator).
            vh = [pp.tile([P, 16, 130], BF, name=f"vh{pr}", tag=f"vh{pr}") for pr in range(2)]

            dst8_of = {"q": q8, "k": k8}

            # ---------- input DMAs (issue all up front; ordered by need) ----
            def dma_w_quarter(nm, q):
                src = blob_d[:, _W_OFF[nm] + q * 2048:_W_OFF[nm] + (q + 1) * 2048]
                nc.sync.dma_start(out=w_sb[nm][q][:],
                                  in_=src.rearrange("p (i o) -> p i o", i=8))

            def dma_x_pair(nm, pr, split=False):
                base = _X_OFF[nm] + pr * 2048
                src = blob_d[:, base:base + 2048].rearrange("p (i o) -> p i o", i=8)
                if split:
                    for hh in range(2):
                        nc.sync.dma_start(
                            out=x_sb[nm][pr][:, hh * 4:(hh + 1) * 4, :],
                            in_=src[:, hh * 4:(hh + 1) * 4, :])
                else:
                    nc.sync.dma_start(out=x_sb[nm][pr][:], in_=src)

            def dma_bias(nm):
                src = blob_d[:, _B_OFF[nm]:_B_OFF[nm] + 16].bitcast(F32)
                nc.sync.dma_start(out=b_sb[nm][:], in_=src)

            dma_w_quarter("q", 0)
            dma_x_pair("q", 0, split=True)
            dma_bias("q")
            dma_w_quarter("q", 1)
            dma_w_quarter("k", 0)
            dma_x_pair("k", 0, split=True)
            dma_bias("k")
            dma_bias("v")
            dma_w_quarter("k", 1)
            dma_w_quarter("q", 2)
            dma_w_quarter("q", 3)
            dma_w_quarter("k", 2)
            dma_w_quarter("k", 3)
            dma_x_pair("q", 1)
            dma_x_pair("k", 1)
            nc.sync.dma_start(out=bo_sb[:], in_=blob_d[0:1, _BO_OFF:_BO_OFF + 1024])
            for q in range(4):
                dma_w_quarter("v", q)
            dma_x_pair("v", 0)
            dma_x_pair("v", 1)
            for hhalf in range(2):
                src = blob_d[:, _WO_OFF + hhalf * 4096:_WO_OFF + (hhalf + 1) * 4096]
                nc.sync.dma_start(out=w_sb["o"][hhalf][:],
                                  in_=src.rearrange("p (i o) -> p i o", i=8))

            nc.gpsimd.memset(ones_bf, 1.0)
            # dead DoubleRow pair halves must be zero (summed into scores)
            for u in range(4):
                nc.gpsimd.memset(q8[u][:, 1, :], 0.0)
                nc.gpsimd.memset(k8[u][:, 1, :], 0.0)
            # strided non-zero memset is invalid ISA on this build: memset a
            # contiguous ones tile and copy it into the strided columns.
            ones_col = pp.tile([P, 16], BF, name="ones_col", tag="ones_col")
            nc.gpsimd.memset(ones_col, 1.0)
            ident = pp.tile([P, P], BF, name="ident", tag="ident")
            make_identity(nc, ident)
            # warm the Act engine's Exp table during the DMA prefix so the
            # first real exp doesn't pay the table load
            scrap = pp.tile([1, P], BF, name="scrap", tag="scrap")
            nc.gpsimd.memset(scrap, 0.0)
            scrap2 = pp.tile([1, P], BF, name="scrap2", tag="scrap2")
            nc.scalar.activation(scrap2[:], scrap[:], EXP)
            # warm the PE p-state during the DMA prefix: ~3us of dummy
            # matmuls on the identity so real work starts at full clock
            warm = nps.tile([P, 512], F32, tag="np", name="warm")
            for wi in range(20):
                nc.tensor.matmul(
                    warm[:, 0:P], lhsT=ident[:, 0:P], rhs=ident[:, 0:P],
                    start=True, stop=True, skip_group_check=True)
            for pr in range(2):
                nc.vector.tensor_copy(out=vh[pr][:, :, 64:65], in_=ones_col[:])
                nc.vector.tensor_copy(out=vh[pr][:, :, 129:130], in_=ones_col[:])

            # ---------- emission helpers ----------
            emitted = set()

            def emit_A(nm, c, pr):
                """Projection chunk: dout block c for pair pr (256 seq cols),
                plus PSUM->SBUF reshape with bias into the head-T tiles.
                q/k land in the fp8 DoubleRow layout; v stays bf16."""
                ps = nps.tile([P, 512], F32, tag="np", name=f"pj_{nm}_{c}_{pr}")
                pj = ps[:, 0:256]
                for i in range(8):
                    nc.tensor.matmul(
                        pj,
                        lhsT=w_sb[nm][c // 2][:, i, (c % 2) * 128:(c % 2) * 128 + 128],
                        rhs=x_sb[nm][pr][:, i, :],
                        start=(i == 0),
                        stop=(i == 7),
                        skip_group_check=True,
                    )
                for pu in range(2):
                    for hh in range(2):
                        j = 2 * c + hh
                        if nm == "v":
                            dst = vhT[pr][pu * 64:pu * 64 + 64, j * P:(j + 1) * P]
                            src = pj[hh * 64:(hh + 1) * 64, pu * P:(pu + 1) * P]
                            bias = b_sb[nm][hh * 64:(hh + 1) * 64, c:c + 1].to_broadcast((64, P))
                            nc.vector.tensor_tensor(out=dst, in0=src, in1=bias, op=ADD)
                        else:
                            dst = dst8_of[nm][2 * pr + pu][:, 0, j * P:(j + 1) * P]
                            src = pj[hh * 64:(hh + 1) * 64, pu * P:(pu + 1) * P]
                            bias = b_sb[nm][hh * 64:(hh + 1) * 64, c:c + 1].to_broadcast((64, P))
                            nc.vector.tensor_tensor(out=dst, in0=src, in1=bias, op=ADD)
                emitted.add(("A", nm, c, pr))

            def emit_VT(pr, idx):
                """Transpose 2 t-chunks of vhT (both units) via PE identity
                matmul + one strided DVE copy per chunk."""
                for tt in range(2 * idx, 2 * idx + 2):
                    tp = nps.tile([P, 1024], BF, tag="np", name=f"vtp_{pr}_{tt}")
                    nc.tensor.transpose(tp[:, 0:P], vhT[pr][:, tt * P:(tt + 1) * P], ident)
                    nc.vector.tensor_copy(
                        out=vh[pr][:, tt, :].rearrange("p (a c) -> p a c", a=2)[:, :, 0:64],
                        in_=tp[:, 0:P].rearrange("p (a c) -> p a c", a=2),
                    )
                emitted.add(("VT", pr, idx))
                if all(("VT", pr, k) in emitted for k in range(8)):
                    emitted.add(("VTall", pr))

            pt_tiles = {}   # (u, sb, tt) -> pt tile
            ctx_tiles = {}  # (u, sb, half) -> PSUM ctx tile [P, 4, 65]
            ctxg_tiles = {}  # (u, sb) -> SBUF [P, 8, 64]
            g_tiles = {}    # (u, c) -> SBUF [P, P]

            def emit_QK(u, sb, tt):
                sc = wps.tile([P, 1024], F32, tag="sc", name=f"sc_{u}_{sb}_{tt}")
                for qq in range(2):
                    s0 = sb * 1024 + qq * 512
                    nc.tensor.matmul(
                        sc[:, qq * 512:(qq + 1) * 512],
                        lhsT=k8[u][:, :, tt * P:(tt + 1) * P],
                        rhs=q8[u][:, :, s0:s0 + 512],
                        start=True,
                        stop=True,
                        perf_mode=DR,
                        skip_group_check=True,
                    )
                pt = ptpool.tile([P, 1024], BF, tag="pt", name=f"pt_{u}_{sb}_{tt}")
                nc.scalar.activation(pt[:], sc[:], EXP, scale=0.125)
                pt_tiles[(u, sb, tt)] = pt
                dbg_pt[(u, sb, tt)] = pt
                emitted.add(("QK", u, sb, tt))

            def emit_C(u, sb, tq):
                """PV for one q-chunk: one sequential accumulation group
                (the interp mis-executes interleaved groups within a tile)."""
                pr, pu = u // 2, u % 2
                if tq % 4 == 0:
                    ctx_tiles[(u, sb, tq // 4)] = cps.tile(
                        [P, 4, 65], F32, tag="ctx", name=f"ctx_{u}_{sb}_{tq // 4}")
                ctx = ctx_tiles[(u, sb, tq // 4)]
                for tt in range(16):
                    nc.tensor.matmul(
                        ctx[:, tq % 4, :],
                        lhsT=pt_tiles[(u, sb, tt)][:, tq * P:(tq + 1) * P],
                        rhs=vh[pr][:, tt, pu * 65:pu * 65 + 65],
                        start=(tt == 0),
                        stop=(tt == 15),
                        skip_group_check=True,
                    )
                if tq == 7:
                    for ttt in range(16):
                        del pt_tiles[(u, sb, ttt)]
                emitted.add(("C", u, sb, tq))

            def emit_SC(u, sb, half):
                """Divide out softmax denominators: ctx PSUM -> ctxg SBUF."""
                if (u, sb) not in ctxg_tiles:
                    ctxg_tiles[(u, sb)] = cgpool.tile(
                        [P, 8, 64], BF, tag="ctxg", name=f"cg_{u}_{sb}")
                ctx = ctx_tiles[(u, sb, half)]
                rec = recpool.tile([P, 4, 1], F32, tag="rec", name=f"rec_{u}_{sb}_{half}")
                nc.vector.reciprocal(rec[:], ctx[:, :, 64:65])
                nc.vector.tensor_tensor(
                    out=ctxg_tiles[(u, sb)][:, half * 4:(half + 1) * 4, :],
                    in0=ctx[:, :, 0:64],
                    in1=rec[:].to_broadcast((P, 4, 64)),
                    op=MUL,
                )
                del ctx_tiles[(u, sb, half)]
                emitted.add(("SC", u, sb, half))

            def emit_GT(u, c):
                sb = c // 4
                lo = (2 * c) % 8
                g = gpool.tile([P, P], BF, tag="g", name=f"g_{u}_{c}")
                tp = cps.tile([P, 520], BF, tag="ctx", name=f"gtp_{u}_{c}")
                nc.tensor.transpose(tp[:, 0:P], ctxg_tiles[(u, sb)][:, lo:lo + 2, :], ident)
                nc.vector.tensor_copy(out=g[:], in_=tp[:, 0:P])
                g_tiles[(u, c)] = g
                emitted.add(("GT", u, c))

            ops_tiles = {}

            def emit_Dmm(u, c):
                if (u, "ops") not in ops_tiles:
                    ops_tiles[(u, "ops")] = [
                        nps.tile([P, 512], F32, tag="np", name=f"out_{u}_{ot}")
                        for ot in range(2)
                    ]
                ops = ops_tiles[(u, "ops")]
                g = g_tiles[(u, c)]
                if c == 0 and BIAS_START:
                    for ot in range(2):
                        nc.tensor.matmul(
                            ops[ot], lhsT=ones_bf[:, :],
                            rhs=bo_sb[:, ot * 512:(ot + 1) * 512],
                            start=True, stop=False, skip_group_check=True)
                for ot in range(2):
                    nc.tensor.matmul(
                        ops[ot],
                        lhsT=g[:],
                        rhs=w_sb["o"][ot][:, c, :],
                        start=(c == 0 and not BIAS_START),
                        stop=(c == 7 and BIAS_START == 1),
                        skip_group_check=True,
                    )
                if c == 7:
                    if not BIAS_START:
                        for ot in range(2):
                            nc.tensor.matmul(
                                ops[ot], lhsT=ones_bf[:, :],
                                rhs=bo_sb[:, ot * 512:(ot + 1) * 512],
                                start=False, stop=True, skip_group_check=True)
                    for ot in range(2):
                        for qtr in range(2):
                            ostg = opool.tile([P, 256], BF, tag="ost",
                                              name=f"ostg_{u}_{ot}_{qtr}")
                            nc.vector.tensor_copy(
                                out=ostg[:], in_=ops[ot][:, qtr * 256:(qtr + 1) * 256])
                            nc.sync.dma_start(
                                out=out_d[u, :, ot * 512 + qtr * 256:ot * 512 + (qtr + 1) * 256],
                                in_=ostg[:])
                    del ops_tiles[(u, "ops")]
                emitted.add(("Dmm", u, c))

            # ---------- work lists ----------
            # fill items: (kind, args, pe_cost_ns, gate) where gate is a
            # predicate on `emitted` that must hold before emission.
            def A_item(nm, c, pr):
                return ("A", (nm, c, pr), 854.0, None)

            def VT_item(pr, idx):
                need = range(idx // 4 * 4, idx // 4 * 4 + 4)
                return ("VT", (pr, idx), 250.0,
                        lambda: all(("A", "v", c, pr) in emitted for c in need))

            def SC_item(u, sb, half):
                return ("SC", (u, sb, half), 30.0,
                        lambda: ("C", u, sb, 4 * half + 3) in emitted)

            def C_item(u, sb, tq):
                pr = u // 2
                return ("C", (u, sb, tq), 433.0,
                        lambda: ("QK", u, sb, 15) in emitted
                        and ("VTall", pr) in emitted
                        and (tq == 0 or ("C", u, sb, tq - 1) in emitted))

            def GT_item(u, c):
                sb = c // 4
                half = ((2 * c) % 8) // 4
                return ("GT", (u, c), 120.0,
                        lambda: ("SC", u, sb, half) in emitted)

            def D_item(u, c):
                return ("Dmm", (u, c), 430.0 if c != 7 else 2200.0,
                        lambda: ("GT", u, c) in emitted)

            fill = deque()
            for c in range(1, 8):
                fill.append(A_item("k", c, 0))
            for c in range(4, 8):
                fill.append(A_item("q", c, 0))
            for c in range(8):
                fill.append(A_item("v", c, 0))
            for idx in range(8):
                fill.append(VT_item(0, idx))

            def add_C_set(u, sb):
                for tq in range(4):
                    fill.append(C_item(u, sb, tq))
                fill.append(SC_item(u, sb, 0))
                for tq in range(4, 8):
                    fill.append(C_item(u, sb, tq))
                fill.append(SC_item(u, sb, 1))

            add_C_set(0, 0)
            for c in range(8):
                fill.append(A_item("q", c, 1))
            add_C_set(0, 1)
            for c in range(8):
                fill.append(GT_item(0, c))
            for c in range(8):
                fill.append(A_item("k", c, 1))
            add_C_set(1, 0)
            for c in range(4):
                fill.append(GT_item(1, c))
            add_C_set(1, 1)
            for c in range(4, 8):
                fill.append(GT_item(1, c))
            for c in range(8):
                fill.append(A_item("v", c, 1))
            for idx in range(8):
                fill.append(VT_item(1, idx))
            for c in range(8):
                fill.append(D_item(0, c))
            add_C_set(2, 0)
            for c in range(4):
                fill.append(GT_item(2, c))
            for c in range(8):
                fill.append(D_item(1, c))
            add_C_set(2, 1)
            for c in range(4, 8):
                fill.append(GT_item(2, c))
            for c in range(4):
                fill.append(D_item(2, c))
            add_C_set(3, 0)
            for c in range(4):
                fill.append(GT_item(3, c))
            for c in range(4, 8):
                fill.append(D_item(2, c))
            for c in range(4):
                fill.append(D_item(3, c))
            add_C_set(3, 1)
            for c in range(4, 8):
                fill.append(GT_item(3, c))
            for c in range(4, 8):
                fill.append(D_item(3, c))

            emit_of = {"A": emit_A, "VT": emit_VT, "SC": emit_SC,
                       "GT": emit_GT, "Dmm": emit_Dmm, "C": emit_C}
            dbg_pt = {}
            _prog_cache["dbg"] = {
                "q8": q8, "k8": k8, "vhT": vhT, "vh": vh,
                "ctxg": ctxg_tiles, "g": g_tiles, "pt": dbg_pt,
            }

            # ---------- prefix: enough projection for the first QK tiles ----
            for c in range(4):
                emit_A("q", c, 0)
            emit_A("k", 0, 0)

            # ---------- paced main loop ----------
            qk_list = [(u, sb, tt) for u in range(4) for sb in range(2)
                       for tt in range(16)]

            # --- clock-model pacing ---
            QK_COST = QK_COST_T
            EXP_COST = EXP_COST_T
            pe_clock = 0.0
            act_clock = 0.0
            exp_done = {}      # qk index -> est completion of its exp
            blk_exp_done = {}  # (u, sb) -> est completion of exp tt=15
            item_cost = {"A": 854.0, "VT": 250.0, "SC": 0.0, "GT": 120.0,
                         "C": 433.0, "Dmm": 430.0}
            ready_time = {}    # optional per-item earliest-pop time

            def item_rt(k):
                """Eligibility of fill[k]: None if gated, else earliest PE time."""
                kind, args, cost, gate = fill[k]
                if gate is not None and not gate():
                    return None
                if kind == "C" and args[2] == 0:
                    return blk_exp_done.get((args[0], args[1]), 0.0)
                return 0.0

            SKIP_W = SKIP_WIN

            def find_eligible(pe_horizon):
                """First eligible item within the skip window; only gated
                C/SC/GT items (cps/DVE only) may be skipped over."""
                for k in range(min(len(fill), SKIP_W)):
                    rt = item_rt(k)
                    if rt is not None and rt <= pe_horizon:
                        return k
                    if fill[k][0] in ("A", "VT", "Dmm"):
                        return None
                return None

            def pop_at(k):
                nonlocal pe_clock
                kind, args, cost, gate = fill[k]
                del fill[k]
                emit_of[kind](*args)
                pe_clock += item_cost.get(kind, cost)

            def qk_ready(u, sb, tt):
                pr = u // 2
                cs = range(4 * sb, 4 * sb + 4)
                if ("A", "k", tt // 2, pr) not in emitted:
                    return False
                return all(("A", "q", c, pr) in emitted for c in cs)

            for i, (u, sb, tt) in enumerate(qk_list):
                while not qk_ready(u, sb, tt):
                    if not fill:
                        raise RuntimeError("fill exhausted before QK deps met")
                    k = find_eligible(float("inf"))
                    if k is None:
                        raise RuntimeError(f"gated fill head {fill[0][0]}{fill[0][1]} blocks QK deps")
                    pop_at(k)
                # fill while the next QK would block on its sc slot
                slot_free = exp_done.get(i - 2, 0.0)
                while fill and pe_clock < slot_free:
                    k = find_eligible(pe_clock + RT_MARGIN)
                    if k is None:
                        break
                    pop_at(k)
                emit_QK(u, sb, tt)
                qk_done = max(pe_clock, slot_free) + QK_COST
                pe_clock = qk_done
                act_clock = max(act_clock, qk_done) + EXP_COST
                exp_done[i] = act_clock
                if tt == 15:
                    blk_exp_done[(u, sb)] = act_clock

            while fill:
                k = find_eligible(float("inf"))
                assert k is not None, f"final fill gate unmet: {fill[0][0]}{fill[0][1]}"
                pop_at(k)

            if _debug_dumps:
                import os
                # probe transposes with position-coded inputs
                tc_a = nc.declare_dram_parameter("tc_a", [64, 2048], BF, isOutput=False)
                tc_b = nc.declare_dram_parameter("tc_b", [64, 2048], BF, isOutput=False)
                tc_c = nc.declare_dram_parameter("tc_c", [P, 128], BF, isOutput=False)
                td_a = nc.declare_dram_parameter("td_a", [P, 16 * 64], BF, isOutput=True)
                td_b = nc.declare_dram_parameter("td_b", [P, 16 * 64], BF, isOutput=True)
                td_c = nc.declare_dram_parameter("td_c", [P, 128], BF, isOutput=True)
                pa = pp.tile([64, 2048], BF, name="pa", tag="pa")
                pb = pp.tile([64, 2048], BF, name="pb", tag="pb")
                pc = pp.tile([P, 2, 64], BF, name="pc", tag="pc")
                ta = pp.tile([P, 16, 64], BF, name="ta", tag="ta")
                tb = pp.tile([P, 16, 64], BF, name="tb", tag="tb")
                tg = pp.tile([P, P], BF, name="tg", tag="tg")
                nc.sync.dma_start(out=pa[:], in_=tc_a[:])
                nc.sync.dma_start(out=pb[:], in_=tc_b[:])
                nc.sync.dma_start(out=pc[:], in_=tc_c.rearrange("p (a b) -> p a b", a=2))
                nc.sync.dma_start(out=ta[:], in_=pa[:], transpose=True)
                nc.sync.dma_start(out=tb[:], in_=pb[:], transpose=True)
                nc.sync.dma_start(out=tg[:], in_=pc[:], transpose=True)
                nc.sync.dma_start(out=td_a.rearrange("p (a b) -> p a b", a=16), in_=ta[:])
                nc.sync.dma_start(out=td_b.rearrange("p (a b) -> p a b", a=16), in_=tb[:])
                nc.sync.dma_start(out=td_c[:], in_=tg[:])
                dbg_vh = nc.declare_dram_parameter("dbg_vh", [P, 16 * 130], BF, isOutput=True)
                dbg_qh = nc.declare_dram_parameter("dbg_qh", [P, S], BF, isOutput=True)
                dbg_kh = nc.declare_dram_parameter("dbg_kh", [P, S], BF, isOutput=True)
                dbg_cg = nc.declare_dram_parameter("dbg_cg", [P, 8 * 64], BF, isOutput=True)
                dbg_g = nc.declare_dram_parameter("dbg_g", [P, P], BF, isOutput=True)
                nc.sync.dma_start(out=dbg_vh.rearrange("p (a b) -> p a b", a=16), in_=vh[0][:])
                nc.sync.dma_start(out=dbg_qh[0:64, :], in_=q8[0][:, 0, :])
                nc.sync.dma_start(out=dbg_kh[0:64, :], in_=k8[0][:, 0, :])
                nc.sync.dma_start(out=dbg_cg.rearrange("p (a b) -> p a b", a=8),
                                  in_=ctxg_tiles[(3, 1)][:])
                nc.sync.dma_start(out=dbg_g[:], in_=g_tiles[(3, 7)][:])

    return nc


def _get_program(_debug_dumps=False):
    key = "nc_dbg" if _debug_dumps else "nc"
    if key not in _prog_cache:
        _prog_cache[key] = _build_program(_debug_dumps)
    return _prog_cache[key]


def _prepare_in_maps(query, key, value, Wq, bq, Wk, bk, Wv, bv, Wo, bo):
    # weight region content: wT.reshape(8,128,1024) transposed so that
    # region [P, 8*256] equals the target SBUF tile [P, 8, 256] bytes.
    def w_region(W):
        wT = np.ascontiguousarray(W.T).astype(_BF16)  # [d_in, d_out]
        return wT.reshape(8, P, D).transpose(1, 0, 2)  # [P, 8, d_out]

    wreg = {"q": w_region(Wq), "k": w_region(Wk), "v": w_region(Wv)}
    woreg = w_region(Wo)
    breg = {
        "q": np.ascontiguousarray(bq.reshape(8, P).T).astype(np.float32),
        "k": np.ascontiguousarray(bk.reshape(8, P).T).astype(np.float32),
        "v": np.ascontiguousarray(bv.reshape(8, P).T).astype(np.float32),
    }
    bo2 = np.asarray(bo).reshape(D).astype(_BF16)

    common = np.zeros((P, BLOB_COLS), _BF16)
    for nm in ("q", "k", "v"):
        for q in range(4):
            common[:, _W_OFF[nm] + q * 2048:_W_OFF[nm] + (q + 1) * 2048] = (
                wreg[nm][:, :, q * 256:(q + 1) * 256].reshape(P, 2048))
        common[:, _B_OFF[nm]:_B_OFF[nm] + 16] = breg[nm].view(_BF16)
    for hh in range(2):
        common[:, _WO_OFF + hh * 4096:_WO_OFF + (hh + 1) * 4096] = (
            woreg[:, :, hh * 512:(hh + 1) * 512].reshape(P, 4096))
    common[0, _BO_OFF:_BO_OFF + 1024] = bo2

    in_maps = []
    for core in range(NCORES):
        units = [core * UPC + k for k in range(UPC)]
        blob = common.copy()
        for nm, full in (("q", query), ("k", key), ("v", value)):
            cols = [
                np.ascontiguousarray(full[u // H, (u % H) * P:(u % H + 1) * P, :].T)
                for u in units
            ]
            xT = np.concatenate(cols, axis=1).astype(_BF16)  # [D, 512]
            xr = xT.reshape(8, P, 512).transpose(1, 0, 2)    # [P, 8, 512]
            for pr in range(2):
                blob[:, _X_OFF[nm] + pr * 2048:_X_OFF[nm] + (pr + 1) * 2048] = (
                    xr[:, :, pr * 256:(pr + 1) * 256].reshape(P, 2048))
        in_maps.append({"blob": blob})
    return in_maps


def kernel(query, key, value, Wq, bq, Wk, bk, Wv, bv, Wo, bo, _trace=False):
    from concourse.bass_utils import run_bass_kernel_spmd

    _install_bir_legalizer()

    query = np.asarray(query, dtype=np.float32)
    key = np.asarray(key, dtype=np.float32)
    value = np.asarray(value, dtype=np.float32)

    nc = _get_program()
    in_maps = _prepare_in_maps(query, key, value,
                               np.asarray(Wq), np.asarray(bq), np.asarray(Wk),
                               np.asarray(bk), np.asarray(Wv), np.asarray(bv),
                               np.asarray(Wo), np.asarray(bo))
    core_ids = list(range(NCORES))
    res = run_bass_kernel_spmd(nc, in_maps, core_ids, trace=_trace)
    _prog_cache["last_results"] = res

    out = np.empty((B, S, D), np.float32)
    for core in range(NCORES):
        o = res.results[core]["out"]
        for k in range(UPC):
            u = core * UPC + k
            out[u // H, (u % H) * P:(u % H + 1) * P, :] = np.asarray(o[k], dtype=np.float32)
    return out

